# revision 18
# baseline (speedup 1.0000x reference)
"""DisparityConv kernel for 8 Trainium2 NeuronCores.

Full inputs: x[8,32,256,512] f32, W[64,32,3,3] f32, bias[64] f32.
Data-parallel over batch: core i computes x[i] -> out[i] [64,256,512].

Per-core pipeline:
  xe (bf16, width-extended by S for the circular roll) prepared host-side.
  Partition layout (j,c): 4 stagger-replicas (XR baked shift j+1) x 32 channels.
  Per shift-group g (delta=4g): |XR[.., w+4g] - X4[.., w]| gives abs-diffs for
  shifts s=4g+j+1 on partition group j. Subs on DVE (TT bf16 2x), abs on
  ScalarE (bf16 2x) -- the measured-optimal split; engines' streams are
  emission-interleaved so psum evictions are not head-of-line blocked.
  Channel mean + shift departition: mask matmul on PE -> psum[(jrep,s), w],
  jrep replicas 0..2 become the kh=0..2 blocks of the conv's K=96 operand.
  psum->Dstage bf16 evictions split 3:5 between DVE (658ns) and ScalarE
  (1.7us) per block; 3 SBUF-SBUF DMAs scatter the three kh blocks onto a
  DIAGONAL ring D3h (block kh of diff row r lands at slot (r+1-kh)%R), so
  conv for output row rr reads the single uniform slot rr%R and gets rows
  rr-1, rr, rr+1 stacked on partitions: 3 dense K=96 matmuls (one per kw,
  rhs w-offset) accumulating in PSUM, col-split by row parity.
  Bias added by DVE on PSUM eviction; bf16 DMA out, host casts to f32.
"""
import sys

sys.path.insert(0, "/opt/trn_rl_repo")

import numpy as np
import ml_dtypes

import concourse.bass as bass  # noqa: F401
import concourse.tile as tile
from concourse import bacc, mybir
from concourse import bass_utils
from concourse import dve_ops
from concourse.dve_ops import DveOp
from concourse.dve_spec import Spec, Src0, Src1, maxx, lower, _has_src1
from concourse.dve_uop import DveOpSpec

F32 = mybir.dt.float32
BF16 = mybir.dt.bfloat16
Alu = mybir.AluOpType
Act = mybir.ActivationFunctionType

B = 8
C = 32
S = 32
O = 64
SG = S // 4
FULL_H, FULL_W = 256, 512
N_CORES = 8

# Per-(block,group) engine schedule, cycled mod len:
# 'v' = DVE sub + ACT abs (3.70us), 'i' = DVE sub + DVE int16 sign-bit-clear
# abs (1.07us, 4x), 'G' = DVE sub + GPSIMD int16 sign-bit-clear abs (~3.5us).
# 1i/3v/4G balances DVE/Scalar/GpSimd at ~19.3us per 8-row block (measured
# costs: sub 2.28, scalar-abs 3.70, dve-and 1.07, psd evict 0.68 scalar,
# bias evict 0.68 scalar-Identity).
ABS_SCHEDULE = "iviv"
# Which of the 8 per-block psum->Dstage evictions run on DVE (rest ScalarE);
# key (rl_half, q). Scalar's psum copy measures 684ns == DVE's, and DVE is
# sub-bound, so all evictions ride ScalarE.
DVE_EVICTS = set()


def _register_abs_diff():
    if "ABS_DIFF_ANT" in dve_ops._SUB_OPCODE_FOR_NAME:
        return dve_ops._ABS_DIFF_ANT_OP
    spec = Spec(
        body=maxx(Src0 - Src1, Src1 - Src0),
        reference=lambda in0, in1, s0, s1, imm2: np.abs(
            in0.astype(np.float32)
            - in1.astype(np.float32).reshape(in0.shape)
        ),
    )
    row = dve_ops._CUSTOM_DVE_ROW_BASE + len(dve_ops.OPS)
    assert row < 0x20
    op = DveOp("ABS_DIFF_ANT", spec, subdim=False, uops_sha={})
    dve_ops._SUB_OPCODE_FOR_NAME["ABS_DIFF_ANT"] = row
    dve_ops.OPS.append(op)
    dve_ops.CUSTOM_DVE_SPECS["ABS_DIFF_ANT"] = spec
    for ver in ("v3",):
        tmp = DveOpSpec(
            name="ABS_DIFF_ANT", opcode=row, uops=lower(spec, ver=ver),
            rd1_en=_has_src1(spec),
        )
        op.uops_sha[ver] = tmp.sha(ver)
    dve_ops._ABS_DIFF_ANT_OP = op
    return op


def _build_nc(H=FULL_H, W=FULL_W, hb=8, R=12, num_devices=N_CORES):
    absd = _register_abs_diff()
    WR = (S - 4) + W
    WE = W + S
    nc = bacc.Bacc("TRN2", target_bir_lowering=False, debug=False,
                   num_devices=num_devices)

    xe = nc.dram_tensor("xe", [C, H, WE], BF16, kind="ExternalInput").ap()
    masks = nc.dram_tensor("masks", [SG, 128, 128], BF16, kind="ExternalInput").ap()
    convw = nc.dram_tensor("convw", [3, 96, O], BF16, kind="ExternalInput").ap()
    bias2 = nc.dram_tensor("bias2", [128, 1], F32, kind="ExternalInput").ap()
    out = nc.dram_tensor("out", [O, H, W], BF16, kind="ExternalOutput").ap()
    out_hv = out.rearrange("o h w -> h o w")

    assert H % hb == 0 and H % 2 == 0
    nblk = H // hb

    with tile.TileContext(nc) as tc:
        with (
            tc.tile_pool(name="const", bufs=1) as constp,
            tc.tile_pool(name="stage", bufs=2) as stagep,
            tc.tile_pool(name="ap", bufs=2 * SG) as apool,
            tc.tile_pool(name="dst", bufs=4) as dstp,
            tc.tile_pool(name="d4p", bufs=1) as d4p,
            tc.tile_pool(name="outp", bufs=4) as outp,
            tc.tile_pool(name="psd", bufs=5, space="PSUM") as psdp,
            tc.tile_pool(name="pso", bufs=3, space="PSUM") as psop,
        ):
            maskT = constp.tile([128, SG * 128], BF16)
            for g in range(SG):
                nc.sync.dma_start(maskT[:, g * 128:(g + 1) * 128], masks[g])
            WT = constp.tile([96, 3 * O], BF16)
            for kw in range(3):
                nc.sync.dma_start(WT[:, kw * O:(kw + 1) * O], convw[kw])
            biasT = constp.tile([128, 1], F32)
            nc.sync.dma_start(biasT[:], bias2[:])

            d3 = d4p.tile([96, R * (W + 2)], BF16)
            d3v = d3.rearrange("p (r w) -> p r w", w=W + 2)
            for sl in range(R):
                nc.vector.memset(d3v[:, sl, :], 0)

            def conv_pair(p0):
                pso = psop.tile([128, W], F32, tag="pso", name=f"pso{p0}")
                for half in (0, 1):
                    for kw in (0, 1, 2):
                        rr = p0 + half
                        nc.tensor.matmul(
                            pso[64 * half:64 * half + 64, :],
                            WT[:, kw * O:kw * O + O],
                            d3v[:, rr % R, kw:kw + W],
                            start=(kw == 0), stop=(kw == 2),
                            tile_position=(0, 64 * half),
                        )
                ot = outp.tile([128, W], BF16)
                # fused bias-add eviction on ScalarE: Identity(psum+bias)
                # measures 677ns -- cheaper than DVE's tensor_single_scalar
                nc.scalar.activation(ot[:], pso[:], Act.Identity,
                                     bias=biasT[:])
                nc.sync.dma_start(out_hv[p0:p0 + 2], ot[:])

            nsched = len(ABS_SCHEDULE)

            def load_block(blk):
                h0 = blk * hb
                x4 = stagep.tile([128, hb * W], BF16, tag="x4",
                                 name=f"x4_{blk}")
                xr = stagep.tile([128, hb * WR], BF16, tag="xr",
                                 name=f"xr_{blk}")
                x4v = x4.rearrange("p (h w) -> p h w", w=W)
                xrv = xr.rearrange("p (h w) -> p h w", w=WR)
                for j in range(4):
                    nc.sync.dma_start(x4v[32 * j:32 * j + 32],
                                      xe[:, h0:h0 + hb, 0:W])
                    nc.sync.dma_start(xrv[32 * j:32 * j + 32],
                                      xe[:, h0:h0 + hb, j + 1:j + 1 + WR])
                return x4v, xrv

            def produce_group(blk, g, x4v, xrv):
                mode = ABS_SCHEDULE[(blk * SG + g) % nsched]
                a = apool.tile([128, hb * W], BF16, tag="a",
                               name=f"a_{blk}_{g}")
                av = a.rearrange("p (h w) -> p h w", w=W)
                in0 = xrv[:, :, 4 * g:4 * g + W]
                nc.vector.tensor_sub(av, in0, x4v)
                if mode == "i":
                    ai = a[:].bitcast(mybir.dt.int16)
                    nc.vector.tensor_single_scalar(
                        ai, ai, 0x7FFF, Alu.bitwise_and)
                elif mode == "G":
                    ai = a[:].bitcast(mybir.dt.int16)
                    nc.gpsimd.tensor_single_scalar(
                        ai, ai, 0x7FFF, Alu.bitwise_and)
                else:
                    nc.scalar.activation(a, a, Act.Abs)
                return av

            stage = load_block(0)
            avs = {g: produce_group(0, g, *stage) for g in range(SG)}
            for blk in range(nblk):
                h0 = blk * hb
                last = blk + 1 >= nblk
                if not last:
                    next_stage = load_block(blk + 1)
                next_avs = {}
                for rl in range(0, hb, 4):
                    half = rl // 4
                    r = h0 + rl + 3
                    due = [p0 for p0 in (r - 9, r - 7) if 0 <= p0 <= H - 4]
                    # four rows' mask-matmul chains interleaved (independent
                    # PSUM banks) so drains overlap fills; the due conv pairs
                    # are threaded INTO the mask stream so any wait on abs
                    # tiles / psum banks is filled with ready conv work (keeps
                    # the PE's HAM window busy -- no >3.4us idle -> no 1.2GHz
                    # throttle)
                    psd4 = [psdp.tile([128, W], F32, tag="psd",
                                      name=f"psd{blk}_{rl}_{q2}")
                            for q2 in range(4)]
                    for g in range(SG):
                        for q in range(4):
                            nc.tensor.matmul(
                                psd4[q], maskT[:, g * 128:(g + 1) * 128],
                                avs[g][:, rl + q, :],
                                start=(g == 0), stop=(g == SG - 1),
                            )
                        if g == 2 and due:
                            conv_pair(due[0])
                        if g == 5 and len(due) > 1:
                            conv_pair(due[1])
                    ds4 = dstp.tile([96, 4 * W], BF16, tag="ds4",
                                    name=f"ds4_{blk}_{rl}")
                    ds4v = ds4.rearrange("p (a w) -> p a w", w=W)

                    def evict(q):
                        if (half, q) in DVE_EVICTS:
                            nc.vector.tensor_copy(ds4v[:, q, :],
                                                  psd4[q][0:96, :])
                        else:
                            nc.scalar.copy(ds4v[:, q, :], psd4[q][0:96, :])

                    # produce next block's groups for this half: 'v' groups
                    # first so ScalarE's abs inputs are ready earliest, then
                    # 'i' groups; evictions after the abs emissions so ScalarE
                    # doesn't head-of-line block on a not-yet-stopped psum
                    gs = sorted(range(4 * half, 4 * half + 4),
                                key=lambda g2: ABS_SCHEDULE[
                                    ((blk + 1) * SG + g2) % nsched] != "v")
                    if not last:
                        for g in gs:
                            next_avs[g] = produce_group(blk + 1, g,
                                                        *next_stage)
                    for k in range(4):
                        evict(k)
                    # batched diagonal-ring scatter: rows r-3..r, per kh
                    # block kh of diff row q lands at slot (q+1-kh)%R
                    r0 = r - 3
                    for kh in range(3):
                        s0 = (r0 + 1 - kh) % R
                        n1 = min(4, R - s0)
                        for (a0, sl0, cnt) in (((0, s0, n1),) if n1 == 4 else
                                               ((0, s0, n1), (n1, 0, 4 - n1))):
                            nc.gpsimd.dma_start(
                                d3v[32 * kh:32 * kh + 32,
                                    sl0:sl0 + cnt, 1:W + 1],
                                ds4v[32 * kh:32 * kh + 32,
                                     a0:a0 + cnt, :])
                avs = next_avs
            # zero the kh=2 slot that would hold (nonexistent) diff row H
            nc.vector.memset(d3v[64:96, (H - 1) % R, 1:W + 1], 0)
            conv_pair(H - 6)
            conv_pair(H - 4)
            conv_pair(H - 2)

    nc.compile()
    return nc


_NC_CACHE = {}


def _get_nc():
    if "nc" not in _NC_CACHE:
        _NC_CACHE["nc"] = _build_nc()
    return _NC_CACHE["nc"]


def host_prep_shared(Wc, bias):
    bf16 = ml_dtypes.bfloat16
    masks = np.zeros((SG, 128, 128), np.float32)
    for g in range(SG):
        for j in range(4):
            for jr in range(4):
                masks[g, 32 * j:32 * j + 32, 32 * jr + 4 * g + j] = 1.0 / C
    masks = masks.astype(bf16)
    # convw[kw, 32*kh + s, o] = Wc[o, s, kh, kw]
    convw = np.ascontiguousarray(
        Wc.transpose(3, 2, 1, 0).reshape(3, 96, O)).astype(bf16)
    bias2 = np.concatenate([bias, bias]).reshape(128, 1).astype(np.float32)
    return masks, convw, bias2


def kernel(x, W, bias, _trace=False, _tmpdir=None):
    """x:[8,32,256,512] f32, W:[64,32,3,3] f32, bias:[64] f32 -> [8,64,256,512]."""
    nc = _get_nc()
    bf16 = ml_dtypes.bfloat16
    masks, convw, bias2 = host_prep_shared(np.asarray(W, np.float32),
                                           np.asarray(bias, np.float32))
    x = np.asarray(x, np.float32)
    xe_all = np.concatenate([x, x[:, :, :, :S]], axis=3).astype(bf16)
    in_maps = [
        {"xe": xe_all[i], "masks": masks, "convw": convw, "bias2": bias2}
        for i in range(N_CORES)
    ]
    kw = {}
    if _trace:
        kw = dict(trace=True, tmpdir=_tmpdir)
    res = bass_utils.run_bass_kernel_spmd(
        nc, in_maps, core_ids=list(range(N_CORES)), **kw)
    out = np.stack([res.results[i]["out"].astype(np.float32)
                    for i in range(N_CORES)], axis=0)
    if _trace:
        kernel.last_exec_time_ns = res.exec_time_ns
        kernel.last_results = res
    return out


# revision 23
# speedup vs baseline: 1.0008x; 1.0008x over previous
"""DisparityConv kernel for 8 Trainium2 NeuronCores.

Full inputs: x[8,32,256,512] f32, W[64,32,3,3] f32, bias[64] f32.
Data-parallel over batch: core i computes x[i] -> out[i] [64,256,512].

Per-core pipeline:
  xe (bf16, width-extended by S for the circular roll) prepared host-side.
  Partition layout (j,c): 4 stagger-replicas (XR baked shift j+1) x 32 channels.
  Per shift-group g (delta=4g): |XR[.., w+4g] - X4[.., w]| gives abs-diffs for
  shifts s=4g+j+1 on partition group j. Subs on DVE (TT bf16 2x), abs on
  ScalarE (bf16 2x) -- the measured-optimal split; engines' streams are
  emission-interleaved so psum evictions are not head-of-line blocked.
  Channel mean + shift departition: mask matmul on PE -> psum[(jrep,s), w],
  jrep replicas 0..2 become the kh=0..2 blocks of the conv's K=96 operand.
  psum->Dstage bf16 evictions split 3:5 between DVE (658ns) and ScalarE
  (1.7us) per block; 3 SBUF-SBUF DMAs scatter the three kh blocks onto a
  DIAGONAL ring D3h (block kh of diff row r lands at slot (r+1-kh)%R), so
  conv for output row rr reads the single uniform slot rr%R and gets rows
  rr-1, rr, rr+1 stacked on partitions: 3 dense K=96 matmuls (one per kw,
  rhs w-offset) accumulating in PSUM, col-split by row parity.
  Bias added by DVE on PSUM eviction; bf16 DMA out, host casts to f32.
"""
import sys

sys.path.insert(0, "/opt/trn_rl_repo")

import numpy as np
import ml_dtypes

import concourse.bass as bass  # noqa: F401
import concourse.tile as tile
from concourse import bacc, mybir
from concourse import bass_utils
from concourse import dve_ops
from concourse.dve_ops import DveOp
from concourse.dve_spec import Spec, Src0, Src1, maxx, lower, _has_src1
from concourse.dve_uop import DveOpSpec

F32 = mybir.dt.float32
BF16 = mybir.dt.bfloat16
Alu = mybir.AluOpType
Act = mybir.ActivationFunctionType

B = 8
C = 32
S = 32
O = 64
SG = S // 4
FULL_H, FULL_W = 256, 512
N_CORES = 8

# Per-(block,group) engine schedule, cycled mod len:
# 'v' = DVE sub + ACT abs (3.70us), 'i' = DVE sub + DVE int16 sign-bit-clear
# abs (1.07us, 4x), 'G' = DVE sub + GPSIMD int16 sign-bit-clear abs (~3.5us).
# 1i/3v/4G balances DVE/Scalar/GpSimd at ~19.3us per 8-row block (measured
# costs: sub 2.28, scalar-abs 3.70, dve-and 1.07, psd evict 0.68 scalar,
# bias evict 0.68 scalar-Identity).
ABS_SCHEDULE = "iviv"
# Which of the 8 per-block psum->Dstage evictions run on DVE (rest ScalarE);
# key (rl_half, q). Scalar's psum copy measures 684ns == DVE's, and DVE is
# sub-bound, so all evictions ride ScalarE.
DVE_EVICTS = set()


def _register_abs_diff():
    if "ABS_DIFF_ANT" in dve_ops._SUB_OPCODE_FOR_NAME:
        return dve_ops._ABS_DIFF_ANT_OP
    spec = Spec(
        body=maxx(Src0 - Src1, Src1 - Src0),
        reference=lambda in0, in1, s0, s1, imm2: np.abs(
            in0.astype(np.float32)
            - in1.astype(np.float32).reshape(in0.shape)
        ),
    )
    row = dve_ops._CUSTOM_DVE_ROW_BASE + len(dve_ops.OPS)
    assert row < 0x20
    op = DveOp("ABS_DIFF_ANT", spec, subdim=False, uops_sha={})
    dve_ops._SUB_OPCODE_FOR_NAME["ABS_DIFF_ANT"] = row
    dve_ops.OPS.append(op)
    dve_ops.CUSTOM_DVE_SPECS["ABS_DIFF_ANT"] = spec
    for ver in ("v3",):
        tmp = DveOpSpec(
            name="ABS_DIFF_ANT", opcode=row, uops=lower(spec, ver=ver),
            rd1_en=_has_src1(spec),
        )
        op.uops_sha[ver] = tmp.sha(ver)
    dve_ops._ABS_DIFF_ANT_OP = op
    return op


def _build_nc(H=FULL_H, W=FULL_W, hb=8, R=12, num_devices=N_CORES):
    absd = _register_abs_diff()
    WR = (S - 4) + W
    WE = W + S
    nc = bacc.Bacc("TRN2", target_bir_lowering=False, debug=False,
                   num_devices=num_devices)

    xe = nc.dram_tensor("xe", [C, H, WE], BF16, kind="ExternalInput").ap()
    masks = nc.dram_tensor("masks", [SG, 128, 128], BF16, kind="ExternalInput").ap()
    convw = nc.dram_tensor("convw", [3, 96, O], BF16, kind="ExternalInput").ap()
    bias2 = nc.dram_tensor("bias2", [128, 1], F32, kind="ExternalInput").ap()
    out = nc.dram_tensor("out", [O, H, W], BF16, kind="ExternalOutput").ap()
    out_hv = out.rearrange("o h w -> h o w")

    assert H % hb == 0 and H % 2 == 0
    nblk = H // hb

    with tile.TileContext(nc) as tc:
        with (
            tc.tile_pool(name="const", bufs=1) as constp,
            tc.tile_pool(name="stage", bufs=2) as stagep,
            tc.tile_pool(name="ap", bufs=2 * SG) as apool,
            tc.tile_pool(name="dst", bufs=4) as dstp,
            tc.tile_pool(name="d4p", bufs=1) as d4p,
            tc.tile_pool(name="outp", bufs=4) as outp,
            tc.tile_pool(name="psd", bufs=5, space="PSUM") as psdp,
            tc.tile_pool(name="pso", bufs=3, space="PSUM") as psop,
        ):
            maskT = constp.tile([128, SG * 128], BF16)
            for g in range(SG):
                nc.sync.dma_start(maskT[:, g * 128:(g + 1) * 128], masks[g])
            WT = constp.tile([96, 3 * O], BF16)
            for kw in range(3):
                nc.sync.dma_start(WT[:, kw * O:(kw + 1) * O], convw[kw])
            biasT = constp.tile([128, 1], F32)
            nc.sync.dma_start(biasT[:], bias2[:])

            d3 = d4p.tile([96, R * (W + 2)], BF16)
            d3v = d3.rearrange("p (r w) -> p r w", w=W + 2)
            for sl in range(R):
                nc.vector.memset(d3v[:, sl, :], 0)

            def conv_pair_mm(p0):
                # matmuls only; bias+store deferred so ScalarE's queue can be
                # ordered [abs, bias, evicts] independent of PE emission order
                pso = psop.tile([128, W], F32, tag="pso", name=f"pso{p0}")
                for half in (0, 1):
                    for kw in (0, 1, 2):
                        rr = p0 + half
                        nc.tensor.matmul(
                            pso[64 * half:64 * half + 64, :],
                            WT[:, kw * O:kw * O + O],
                            d3v[:, rr % R, kw:kw + W],
                            start=(kw == 0), stop=(kw == 2),
                            tile_position=(0, 64 * half),
                        )
                return (p0, pso)

            def conv_bias_out(pending):
                for p0, pso in pending:
                    ot = outp.tile([128, W], BF16)
                    # fused bias-add eviction on ScalarE: Identity(psum+bias)
                    # measures 677ns -- cheaper than DVE tensor_single_scalar
                    nc.scalar.activation(ot[:], pso[:], Act.Identity,
                                         bias=biasT[:])
                    nc.sync.dma_start(out_hv[p0:p0 + 2], ot[:])
                pending.clear()

            nsched = len(ABS_SCHEDULE)

            def load_block(blk):
                h0 = blk * hb
                x4 = stagep.tile([128, hb * W], BF16, tag="x4",
                                 name=f"x4_{blk}")
                xr = stagep.tile([128, hb * WR], BF16, tag="xr",
                                 name=f"xr_{blk}")
                x4v = x4.rearrange("p (h w) -> p h w", w=W)
                xrv = xr.rearrange("p (h w) -> p h w", w=WR)
                for j in range(4):
                    nc.sync.dma_start(x4v[32 * j:32 * j + 32],
                                      xe[:, h0:h0 + hb, 0:W])
                    nc.sync.dma_start(xrv[32 * j:32 * j + 32],
                                      xe[:, h0:h0 + hb, j + 1:j + 1 + WR])
                return x4v, xrv

            def produce_group(blk, g, x4v, xrv):
                mode = ABS_SCHEDULE[(blk * SG + g) % nsched]
                a = apool.tile([128, hb * W], BF16, tag="a",
                               name=f"a_{blk}_{g}")
                av = a.rearrange("p (h w) -> p h w", w=W)
                in0 = xrv[:, :, 4 * g:4 * g + W]
                nc.vector.tensor_sub(av, in0, x4v)
                if mode == "i":
                    ai = a[:].bitcast(mybir.dt.int16)
                    nc.vector.tensor_single_scalar(
                        ai, ai, 0x7FFF, Alu.bitwise_and)
                elif mode == "G":
                    ai = a[:].bitcast(mybir.dt.int16)
                    nc.gpsimd.tensor_single_scalar(
                        ai, ai, 0x7FFF, Alu.bitwise_and)
                else:
                    nc.scalar.activation(a, a, Act.Abs)
                return av

            stage = load_block(0)
            avs = {g: produce_group(0, g, *stage) for g in range(SG)}
            pending_conv = []
            for blk in range(nblk):
                h0 = blk * hb
                last = blk + 1 >= nblk
                if not last:
                    next_stage = load_block(blk + 1)
                next_avs = {}
                for rl in range(0, hb, 4):
                    half = rl // 4
                    r = h0 + rl + 3
                    due = [p0 for p0 in (r - 9, r - 7) if 0 <= p0 <= H - 4]
                    # four rows' mask-matmul chains interleaved (independent
                    # PSUM banks) so drains overlap fills; the due conv pairs
                    # are threaded INTO the mask stream so any wait on abs
                    # tiles / psum banks is filled with ready conv work (keeps
                    # the PE's HAM window busy -- no >3.4us idle -> no 1.2GHz
                    # throttle)
                    psd4 = [psdp.tile([128, W], F32, tag="psd",
                                      name=f"psd{blk}_{rl}_{q2}")
                            for q2 in range(4)]
                    for g in range(SG):
                        for q in range(4):
                            nc.tensor.matmul(
                                psd4[q], maskT[:, g * 128:(g + 1) * 128],
                                avs[g][:, rl + q, :],
                                start=(g == 0), stop=(g == SG - 1),
                            )
                        if g == 2 and due:
                            pending_conv.append(conv_pair_mm(due[0]))
                        if g == 5 and len(due) > 1:
                            pending_conv.append(conv_pair_mm(due[1]))
                    ds4 = dstp.tile([96, 4 * W], BF16, tag="ds4",
                                    name=f"ds4_{blk}_{rl}")
                    ds4v = ds4.rearrange("p (a w) -> p a w", w=W)

                    def evict(q):
                        if (half, q) in DVE_EVICTS:
                            nc.vector.tensor_copy(ds4v[:, q, :],
                                                  psd4[q][0:96, :])
                        else:
                            nc.scalar.copy(ds4v[:, q, :], psd4[q][0:96, :])

                    # produce next block's groups for this half: 'v' groups
                    # first so ScalarE's abs inputs are ready earliest, then
                    # 'i' groups; evictions after the abs emissions so ScalarE
                    # doesn't head-of-line block on a not-yet-stopped psum
                    gs = sorted(range(4 * half, 4 * half + 4),
                                key=lambda g2: ABS_SCHEDULE[
                                    ((blk + 1) * SG + g2) % nsched] != "v")
                    if not last:
                        for g in gs:
                            next_avs[g] = produce_group(blk + 1, g,
                                                        *next_stage)
                    conv_bias_out(pending_conv)
                    for k in range(4):
                        evict(k)
                    # batched diagonal-ring scatter: rows r-3..r, per kh
                    # block kh of diff row q lands at slot (q+1-kh)%R
                    r0 = r - 3
                    for kh in range(3):
                        s0 = (r0 + 1 - kh) % R
                        n1 = min(4, R - s0)
                        for (a0, sl0, cnt) in (((0, s0, n1),) if n1 == 4 else
                                               ((0, s0, n1), (n1, 0, 4 - n1))):
                            nc.gpsimd.dma_start(
                                d3v[32 * kh:32 * kh + 32,
                                    sl0:sl0 + cnt, 1:W + 1],
                                ds4v[32 * kh:32 * kh + 32,
                                     a0:a0 + cnt, :])
                avs = next_avs
            # zero the kh=2 slot that would hold (nonexistent) diff row H
            nc.vector.memset(d3v[64:96, (H - 1) % R, 1:W + 1], 0)
            pending_conv.append(conv_pair_mm(H - 6))
            pending_conv.append(conv_pair_mm(H - 4))
            conv_bias_out(pending_conv)
            pending_conv.append(conv_pair_mm(H - 2))
            conv_bias_out(pending_conv)

    nc.compile()
    return nc


_NC_CACHE = {}


def _get_nc():
    if "nc" not in _NC_CACHE:
        _NC_CACHE["nc"] = _build_nc()
    return _NC_CACHE["nc"]


def host_prep_shared(Wc, bias):
    bf16 = ml_dtypes.bfloat16
    masks = np.zeros((SG, 128, 128), np.float32)
    for g in range(SG):
        for j in range(4):
            for jr in range(4):
                masks[g, 32 * j:32 * j + 32, 32 * jr + 4 * g + j] = 1.0 / C
    masks = masks.astype(bf16)
    # convw[kw, 32*kh + s, o] = Wc[o, s, kh, kw]
    convw = np.ascontiguousarray(
        Wc.transpose(3, 2, 1, 0).reshape(3, 96, O)).astype(bf16)
    bias2 = np.concatenate([bias, bias]).reshape(128, 1).astype(np.float32)
    return masks, convw, bias2


def kernel(x, W, bias, _trace=False, _tmpdir=None):
    """x:[8,32,256,512] f32, W:[64,32,3,3] f32, bias:[64] f32 -> [8,64,256,512]."""
    nc = _get_nc()
    bf16 = ml_dtypes.bfloat16
    masks, convw, bias2 = host_prep_shared(np.asarray(W, np.float32),
                                           np.asarray(bias, np.float32))
    x = np.asarray(x, np.float32)
    xe_all = np.concatenate([x, x[:, :, :, :S]], axis=3).astype(bf16)
    in_maps = [
        {"xe": xe_all[i], "masks": masks, "convw": convw, "bias2": bias2}
        for i in range(N_CORES)
    ]
    kw = {}
    if _trace:
        kw = dict(trace=True, tmpdir=_tmpdir)
    res = bass_utils.run_bass_kernel_spmd(
        nc, in_maps, core_ids=list(range(N_CORES)), **kw)
    out = np.stack([res.results[i]["out"].astype(np.float32)
                    for i in range(N_CORES)], axis=0)
    if _trace:
        kernel.last_exec_time_ns = res.exec_time_ns
        kernel.last_results = res
    return out


# revision 24
# speedup vs baseline: 1.0624x; 1.0616x over previous
"""DisparityConv kernel for 8 Trainium2 NeuronCores.

Full inputs: x[8,32,256,512] f32, W[64,32,3,3] f32, bias[64] f32.
Data-parallel over batch: core i computes x[i] -> out[i] [64,256,512].

Per-core pipeline:
  xe (bf16, width-extended by S for the circular roll) prepared host-side.
  Partition layout (j,c): 4 stagger-replicas (XR baked shift j+1) x 32 channels.
  Per shift-group g (delta=4g): |XR[.., w+4g] - X4[.., w]| gives abs-diffs for
  shifts s=4g+j+1 on partition group j. Subs on DVE (TT bf16 2x), abs on
  ScalarE (bf16 2x) -- the measured-optimal split; engines' streams are
  emission-interleaved so psum evictions are not head-of-line blocked.
  Channel mean + shift departition: mask matmul on PE -> psum[(jrep,s), w],
  jrep replicas 0..2 become the kh=0..2 blocks of the conv's K=96 operand.
  psum->Dstage bf16 evictions split 3:5 between DVE (658ns) and ScalarE
  (1.7us) per block; 3 SBUF-SBUF DMAs scatter the three kh blocks onto a
  DIAGONAL ring D3h (block kh of diff row r lands at slot (r+1-kh)%R), so
  conv for output row rr reads the single uniform slot rr%R and gets rows
  rr-1, rr, rr+1 stacked on partitions: 3 dense K=96 matmuls (one per kw,
  rhs w-offset) accumulating in PSUM, col-split by row parity.
  Bias added by DVE on PSUM eviction; bf16 DMA out, host casts to f32.
"""
import sys

sys.path.insert(0, "/opt/trn_rl_repo")

import numpy as np
import ml_dtypes

import concourse.bass as bass  # noqa: F401
import concourse.tile as tile
from concourse import bacc, mybir
from concourse import bass_utils
from concourse import dve_ops
from concourse.dve_ops import DveOp
from concourse.dve_spec import Spec, Src0, Src1, maxx, lower, _has_src1
from concourse.dve_uop import DveOpSpec

F32 = mybir.dt.float32
BF16 = mybir.dt.bfloat16
Alu = mybir.AluOpType
Act = mybir.ActivationFunctionType

B = 8
C = 32
S = 32
O = 64
SG = S // 4
FULL_H, FULL_W = 256, 512
N_CORES = 8

# Per-(block,group) engine schedule, cycled mod len:
# 'v' = DVE sub + ACT abs (3.70us), 'i' = DVE sub + DVE int16 sign-bit-clear
# abs (1.07us, 4x), 'G' = DVE sub + GPSIMD int16 sign-bit-clear abs (~3.5us).
# 1i/3v/4G balances DVE/Scalar/GpSimd at ~19.3us per 8-row block (measured
# costs: sub 2.28, scalar-abs 3.70, dve-and 1.07, psd evict 0.68 scalar,
# bias evict 0.68 scalar-Identity).
ABS_SCHEDULE = "iviv"
# Which of the 8 per-block psum->Dstage evictions run on DVE (rest ScalarE);
# key (rl_half, q). Scalar's psum copy measures 684ns == DVE's, and DVE is
# sub-bound, so all evictions ride ScalarE.
DVE_EVICTS = set()


def _register_abs_diff():
    if "ABS_DIFF_ANT" in dve_ops._SUB_OPCODE_FOR_NAME:
        return dve_ops._ABS_DIFF_ANT_OP
    spec = Spec(
        body=maxx(Src0 - Src1, Src1 - Src0),
        reference=lambda in0, in1, s0, s1, imm2: np.abs(
            in0.astype(np.float32)
            - in1.astype(np.float32).reshape(in0.shape)
        ),
    )
    row = dve_ops._CUSTOM_DVE_ROW_BASE + len(dve_ops.OPS)
    assert row < 0x20
    op = DveOp("ABS_DIFF_ANT", spec, subdim=False, uops_sha={})
    dve_ops._SUB_OPCODE_FOR_NAME["ABS_DIFF_ANT"] = row
    dve_ops.OPS.append(op)
    dve_ops.CUSTOM_DVE_SPECS["ABS_DIFF_ANT"] = spec
    for ver in ("v3",):
        tmp = DveOpSpec(
            name="ABS_DIFF_ANT", opcode=row, uops=lower(spec, ver=ver),
            rd1_en=_has_src1(spec),
        )
        op.uops_sha[ver] = tmp.sha(ver)
    dve_ops._ABS_DIFF_ANT_OP = op
    return op


def _build_nc(H=FULL_H, W=FULL_W, hb=8, R=12, num_devices=N_CORES):
    absd = _register_abs_diff()
    WR = (S - 4) + W
    WE = W + S
    nc = bacc.Bacc("TRN2", target_bir_lowering=False, debug=False,
                   num_devices=num_devices)

    xe = nc.dram_tensor("xe", [C, H, WE], BF16, kind="ExternalInput").ap()
    masks = nc.dram_tensor("masks", [SG, 128, 128], BF16, kind="ExternalInput").ap()
    convw = nc.dram_tensor("convw", [3, 96, O], BF16, kind="ExternalInput").ap()
    bias2 = nc.dram_tensor("bias2", [128, 1], F32, kind="ExternalInput").ap()
    out = nc.dram_tensor("out", [O, H, W], BF16, kind="ExternalOutput").ap()
    out_hv = out.rearrange("o h w -> h o w")

    assert H % hb == 0 and H % 2 == 0
    nblk = H // hb

    with tile.TileContext(nc) as tc:
        with (
            tc.tile_pool(name="const", bufs=1) as constp,
            tc.tile_pool(name="stage", bufs=2) as stagep,
            tc.tile_pool(name="ap", bufs=2 * SG) as apool,
            tc.tile_pool(name="dst", bufs=4) as dstp,
            tc.tile_pool(name="d4p", bufs=1) as d4p,
            tc.tile_pool(name="outp", bufs=4) as outp,
            tc.tile_pool(name="psd", bufs=5, space="PSUM") as psdp,
            tc.tile_pool(name="pso", bufs=3, space="PSUM") as psop,
        ):
            maskT = constp.tile([128, SG * 128], BF16)
            for g in range(SG):
                nc.sync.dma_start(maskT[:, g * 128:(g + 1) * 128], masks[g])
            WT = constp.tile([96, 3 * O], BF16)
            for kw in range(3):
                nc.sync.dma_start(WT[:, kw * O:(kw + 1) * O], convw[kw])
            biasT = constp.tile([128, 1], F32)
            nc.sync.dma_start(biasT[:], bias2[:])

            d3 = d4p.tile([96, R * (W + 2)], BF16)
            d3v = d3.rearrange("p (r w) -> p r w", w=W + 2)
            for sl in range(R):
                nc.vector.memset(d3v[:, sl, :], 0)

            def conv_pair_mm(p0):
                # matmuls only; bias+store deferred so ScalarE's queue can be
                # ordered [abs, bias, evicts] independent of PE emission order
                pso = psop.tile([128, W], F32, tag="pso", name=f"pso{p0}")
                for half in (0, 1):
                    for kw in (0, 1, 2):
                        rr = p0 + half
                        nc.tensor.matmul(
                            pso[64 * half:64 * half + 64, :],
                            WT[:, kw * O:kw * O + O],
                            d3v[:, rr % R, kw:kw + W],
                            start=(kw == 0), stop=(kw == 2),
                            tile_position=(0, 64 * half),
                        )
                return (p0, pso)

            def conv_bias_out(pending):
                for p0, pso in pending:
                    ot = outp.tile([128, W], BF16)
                    # fused bias-add eviction on ScalarE: Identity(psum+bias)
                    # measures 677ns -- cheaper than DVE tensor_single_scalar
                    nc.scalar.activation(ot[:], pso[:], Act.Identity,
                                         bias=biasT[:])
                    nc.sync.dma_start(out_hv[p0:p0 + 2], ot[:])
                pending.clear()

            nsched = len(ABS_SCHEDULE)

            def load_block(blk):
                h0 = blk * hb
                x4 = stagep.tile([128, hb * W], BF16, tag="x4",
                                 name=f"x4_{blk}")
                xr = stagep.tile([128, hb * WR], BF16, tag="xr",
                                 name=f"xr_{blk}")
                x4v = x4.rearrange("p (h w) -> p h w", w=W)
                xrv = xr.rearrange("p (h w) -> p h w", w=WR)
                for j in range(4):
                    nc.sync.dma_start(x4v[32 * j:32 * j + 32],
                                      xe[:, h0:h0 + hb, 0:W])
                    nc.sync.dma_start(xrv[32 * j:32 * j + 32],
                                      xe[:, h0:h0 + hb, j + 1:j + 1 + WR])
                return x4v, xrv

            def produce_group(blk, g, x4v, xrv):
                mode = ABS_SCHEDULE[(blk * SG + g) % nsched]
                a = apool.tile([128, hb * W], BF16, tag="a",
                               name=f"a_{blk}_{g}")
                av = a.rearrange("p (h w) -> p h w", w=W)
                in0 = xrv[:, :, 4 * g:4 * g + W]
                nc.vector.tensor_sub(av, in0, x4v)
                if mode == "i":
                    ai = a[:].bitcast(mybir.dt.int16)
                    nc.vector.tensor_single_scalar(
                        ai, ai, 0x7FFF, Alu.bitwise_and)
                elif mode == "G":
                    ai = a[:].bitcast(mybir.dt.int16)
                    nc.gpsimd.tensor_single_scalar(
                        ai, ai, 0x7FFF, Alu.bitwise_and)
                else:
                    nc.scalar.activation(a, a, Act.Abs)
                return av

            stage = load_block(0)
            avs = {g: produce_group(0, g, *stage) for g in range(SG)}
            pending_conv = []
            for blk in range(nblk):
                h0 = blk * hb
                last = blk + 1 >= nblk
                if not last:
                    next_stage = load_block(blk + 1)
                next_avs = {}
                for rl in range(0, hb, 4):
                    half = rl // 4
                    r = h0 + rl + 3
                    due = [p0 for p0 in (r - 9, r - 7) if 0 <= p0 <= H - 4]
                    # four rows' mask-matmul chains interleaved (independent
                    # PSUM banks) so drains overlap fills; the due conv pairs
                    # are threaded INTO the mask stream so any wait on abs
                    # tiles / psum banks is filled with ready conv work (keeps
                    # the PE's HAM window busy -- no >3.4us idle -> no 1.2GHz
                    # throttle)
                    # mask matmuls in q-PAIRS: chains (q0,q1) run their full
                    # 8-g accumulation first and STOP at ~3.5us, (q2,q3) at
                    # ~7us -- staggered psum-bank release so the next rl's
                    # burst never starves for banks (the v5-v7 critical path)
                    psd4 = [psdp.tile([128, W], F32, tag="psd",
                                      name=f"psd{blk}_{rl}_{q2}")
                            for q2 in range(4)]
                    for qp in ((0, 1), (2, 3)):
                        for g in range(SG):
                            for q in qp:
                                nc.tensor.matmul(
                                    psd4[q], maskT[:, g * 128:(g + 1) * 128],
                                    avs[g][:, rl + q, :],
                                    start=(g == 0), stop=(g == SG - 1),
                                )
                    for p0 in due:
                        pending_conv.append(conv_pair_mm(p0))
                    ds4 = dstp.tile([96, 4 * W], BF16, tag="ds4",
                                    name=f"ds4_{blk}_{rl}")
                    ds4v = ds4.rearrange("p (a w) -> p a w", w=W)

                    def evict(q):
                        if (half, q) in DVE_EVICTS:
                            nc.vector.tensor_copy(ds4v[:, q, :],
                                                  psd4[q][0:96, :])
                        else:
                            nc.scalar.copy(ds4v[:, q, :], psd4[q][0:96, :])

                    # ScalarE queue order [e0, e1, abs, e2, e3, abs, bias]:
                    # evictions as soon as their staggered stops land, abs
                    # filling the space between, bias last (pso has slack).
                    # DVE order: 'v' subs first so abs inputs are ready early.
                    gs = sorted(range(4 * half, 4 * half + 4),
                                key=lambda g2: ABS_SCHEDULE[
                                    ((blk + 1) * SG + g2) % nsched] != "v")
                    vgs = [g for g in gs if ABS_SCHEDULE[
                        ((blk + 1) * SG + g) % nsched] == "v"]
                    igs = [g for g in gs if g not in vgs]
                    evict(0)
                    evict(1)
                    if not last and vgs:
                        next_avs[vgs[0]] = produce_group(blk + 1, vgs[0],
                                                         *next_stage)
                    evict(2)
                    evict(3)
                    if not last:
                        for g in vgs[1:]:
                            next_avs[g] = produce_group(blk + 1, g,
                                                        *next_stage)
                        for g in igs:
                            next_avs[g] = produce_group(blk + 1, g,
                                                        *next_stage)
                    conv_bias_out(pending_conv)
                    # batched diagonal-ring scatter: rows r-3..r, per kh
                    # block kh of diff row q lands at slot (q+1-kh)%R
                    r0 = r - 3
                    for kh in range(3):
                        s0 = (r0 + 1 - kh) % R
                        n1 = min(4, R - s0)
                        for (a0, sl0, cnt) in (((0, s0, n1),) if n1 == 4 else
                                               ((0, s0, n1), (n1, 0, 4 - n1))):
                            nc.gpsimd.dma_start(
                                d3v[32 * kh:32 * kh + 32,
                                    sl0:sl0 + cnt, 1:W + 1],
                                ds4v[32 * kh:32 * kh + 32,
                                     a0:a0 + cnt, :])
                avs = next_avs
            # zero the kh=2 slot that would hold (nonexistent) diff row H
            nc.vector.memset(d3v[64:96, (H - 1) % R, 1:W + 1], 0)
            pending_conv.append(conv_pair_mm(H - 6))
            pending_conv.append(conv_pair_mm(H - 4))
            conv_bias_out(pending_conv)
            pending_conv.append(conv_pair_mm(H - 2))
            conv_bias_out(pending_conv)

    nc.compile()
    return nc


_NC_CACHE = {}


def _get_nc():
    if "nc" not in _NC_CACHE:
        _NC_CACHE["nc"] = _build_nc()
    return _NC_CACHE["nc"]


def host_prep_shared(Wc, bias):
    bf16 = ml_dtypes.bfloat16
    masks = np.zeros((SG, 128, 128), np.float32)
    for g in range(SG):
        for j in range(4):
            for jr in range(4):
                masks[g, 32 * j:32 * j + 32, 32 * jr + 4 * g + j] = 1.0 / C
    masks = masks.astype(bf16)
    # convw[kw, 32*kh + s, o] = Wc[o, s, kh, kw]
    convw = np.ascontiguousarray(
        Wc.transpose(3, 2, 1, 0).reshape(3, 96, O)).astype(bf16)
    bias2 = np.concatenate([bias, bias]).reshape(128, 1).astype(np.float32)
    return masks, convw, bias2


def kernel(x, W, bias, _trace=False, _tmpdir=None):
    """x:[8,32,256,512] f32, W:[64,32,3,3] f32, bias:[64] f32 -> [8,64,256,512]."""
    nc = _get_nc()
    bf16 = ml_dtypes.bfloat16
    masks, convw, bias2 = host_prep_shared(np.asarray(W, np.float32),
                                           np.asarray(bias, np.float32))
    x = np.asarray(x, np.float32)
    xe_all = np.concatenate([x, x[:, :, :, :S]], axis=3).astype(bf16)
    in_maps = [
        {"xe": xe_all[i], "masks": masks, "convw": convw, "bias2": bias2}
        for i in range(N_CORES)
    ]
    kw = {}
    if _trace:
        kw = dict(trace=True, tmpdir=_tmpdir)
    res = bass_utils.run_bass_kernel_spmd(
        nc, in_maps, core_ids=list(range(N_CORES)), **kw)
    out = np.stack([res.results[i]["out"].astype(np.float32)
                    for i in range(N_CORES)], axis=0)
    if _trace:
        kernel.last_exec_time_ns = res.exec_time_ns
        kernel.last_results = res
    return out
